# revision 14
# baseline (speedup 1.0000x reference)
"""Trainium2 Bass kernel for 2D attention with relative-position augmentation.

Problem shapes (hardcoded): inputs [8, 32, 32, 768] fp32 (q|k|v packed on the
channel axis, 256 each), key_rel_w/key_rel_h [63, 32] fp32.
Output: [8, 32, 32, 256] fp32.

Sharding: data-parallel over batch - core b gets batch b (8 cores, no
collectives needed).

Per-core math (N = 32*32 = 1024 tokens, 8 heads, head dim 32):
  L[n, m] = Q[n].K[m] + qdw[n, y2(m)-y(n)+31] + qdh[n, x2(m)-x(n)+31]
  out[n]  = softmax_m(L[n, :] / sqrt(32)) @ V
where qdw = Q @ key_rel_w^T, qdh = Q @ key_rel_h^T and n=(x,y), m=(x2,y2).

Kernel formulation (v2):
  * L^T (m on partitions, n on free dim) so the attention matmul consumes
    P^T = exp(L^T) directly as the stationary operand.
  * Relative-logit terms folded into the SAME matmul as Q.K by extending the
    contraction dim from 32 to 96 (rows 32:64 one-hot Aw / Bw, 64:96 Ah / Bh).
    The Bw/Bh rows (partition-shifted qdw^T/qdh^T) are produced by per-y
    matmuls with shifted free-slices of the rel tables as stationary; four
    32x32 array tiles (2 row-groups x 2 col-groups) run concurrently.
  * Softmax skips max-subtraction (logits are small); 1/sqrt(32) is folded
    into the exponential.
  * exp() is split between the Scalar engine (exact table exp) and the Vector
    engine (Schraudolph bit-trick: bits_i16 = round(x*A + B) reinterpreted as
    bf16 ~= exp(x*ES) with +-3% sawtooth error). The ACT share carries a
    bias = ln(mean sawtooth ratio) so both shares have the same mean weight
    scale and the softmax ratio cancels the systematic part.
  * Row sums via a ones column appended to V inside the AV matmul;
    normalization is one reciprocal + one broadcast tensor_tensor multiply.
"""

import numpy as np

import concourse.bacc as bacc
import concourse.mybir as mybir
from concourse.tile import TileContext
from concourse.bass_utils import run_bass_kernel_spmd

F32 = mybir.dt.float32
BF16 = mybir.dt.bfloat16
I16 = mybir.dt.int16
I32 = mybir.dt.int32
AF = mybir.ActivationFunctionType
ALU = mybir.AluOpType

N_CORES = 8
N = 1024          # tokens per batch (32 x 32)
NH = 8            # heads
EXP_SCALE = float(1.0 / np.sqrt(32.0))

# Schraudolph constants: bits16 = round(x * SCH_A + SCH_B); bf16(bits16)
# ~= exp(x * EXP_SCALE) * (1 + eps), eps in +-3.0% sawtooth.
SCH_SIGMA = 5.6
SCH_A = float(128.0 * np.log2(np.e) * EXP_SCALE)
SCH_B = float(16256.0 - SCH_SIGMA)
# mean of (1+f)*2^-f over f~U[0,1) is 1.0406844, times 2^(-sigma/128):
ACT_BIAS = float(np.log(1.0406844 * 2.0 ** (-SCH_SIGMA / 128.0)))

# which l_ps tiles (i in 0..8) go to the Vector engine's Schraudolph exp,
# per head (error grows ~sqrt(share); keep <= 3/8 for the 2e-2 gate).
DVE_EXP = {}
DVE_EXP_DEFAULT = (1, 4, 6)

_CACHE = {}


def _emit(tc, x, rw, rh, out):
    nc = tc.nc

    with tc.tile_pool(name="big", bufs=1) as big, \
         tc.tile_pool(name="dram", bufs=1, space="DRAM") as dram, \
         tc.tile_pool(name="psp", bufs=4, space="PSUM") as psp, \
         tc.tile_pool(name="ptp", bufs=17) as ptp, \
         tc.tile_pool(name="outp", bufs=4) as outp:

        # ================= setup =================
        # natural-layout loads: n = t*128 + p
        xn = x.rearrange("(t p) c -> p t c", p=128)
        xq = big.tile([128, 2048], F32, name="xq")
        xk = big.tile([128, 2048], F32, name="xk")
        xv = big.tile([128, 2048], F32, name="xv")
        nc.sync.dma_start(out=xq[:].rearrange("p (t c) -> p t c", c=256),
                          in_=xn[:, :, 0:256])
        nc.sync.dma_start(out=xk[:].rearrange("p (t c) -> p t c", c=256),
                          in_=xn[:, :, 256:512])
        nc.sync.dma_start(out=xv[:].rearrange("p (t c) -> p t c", c=256),
                          in_=xn[:, :, 512:768])

        # Q,K: DVE cast fp32->bf16, store to HBM, xbar transpose-load
        qbf = big.tile([128, 2048], BF16, name="qbf")
        kbf = big.tile([128, 2048], BF16, name="kbf")
        nc.vector.tensor_copy(qbf[:], xq[:])
        nc.vector.tensor_copy(kbf[:], xk[:])
        qbf_d = dram.tile([N, 256], BF16, name="qbf_d")
        kbf_d = dram.tile([N, 256], BF16, name="kbf_d")
        nc.sync.dma_start(out=qbf_d[:].rearrange("(t p) c -> p t c", p=128),
                          in_=qbf[:].rearrange("p (t c) -> p t c", c=256))
        nc.sync.dma_start(out=kbf_d[:].rearrange("(t p) c -> p t c", p=128),
                          in_=kbf[:].rearrange("p (t c) -> p t c", c=256))
        qt0 = big.tile([128, N], BF16, name="qt0")
        qt1 = big.tile([128, N], BF16, name="qt1")
        kt0 = big.tile([128, N], BF16, name="kt0")
        kt1 = big.tile([128, N], BF16, name="kt1")
        nc.sync.dma_start(out=qt0[:], in_=qbf_d[:, 0:128], transpose=True)
        nc.sync.dma_start(out=qt1[:], in_=qbf_d[:, 128:256], transpose=True)
        nc.sync.dma_start(out=kt0[:], in_=kbf_d[:, 0:128], transpose=True)
        nc.sync.dma_start(out=kt1[:], in_=kbf_d[:, 128:256], transpose=True)


        # rel tables -> rt128 [128, 128] bf16, the 32-row table replicated at
        # 4 partition offsets (row-groups).  cols 0:63 w-table^T (col r =
        # rel_w[r, :]), col 63 zero; cols 64:127 h-table^T, col 127 zero.
        rel4 = big.tile([32, 128], F32, name="rel4")
        nc.vector.memset(rel4[:, :], 0.0)
        nc.sync.dma_start(out=rel4[0:32, 0:32], in_=rw[0:32, :])
        nc.sync.dma_start(out=rel4[0:31, 32:64], in_=rw[32:63, :])
        nc.sync.dma_start(out=rel4[0:32, 64:96], in_=rh[0:32, :])
        nc.sync.dma_start(out=rel4[0:31, 96:128], in_=rh[32:63, :])
        rtf = big.tile([32, 128], F32, name="rtf")
        nc.vector.transpose(rtf[:, :], rel4[:, :])  # 4x 32x32 block transpose
        rt128 = big.tile([128, 128], BF16, name="rt128")
        nc.vector.tensor_copy(rt128[0:32, :], rtf[:])
        for k in range(1, 4):
            nc.sync.dma_start(out=rt128[32 * k:32 * k + 32, :], in_=rt128[0:32, :])

        # one-hot rows oh2 [64, 1024]: rows 0:32 Aw[y',m]=[m%32==y'], rows
        # 32:64 Ah[x',m]=[m//32==x']
        oh2 = big.tile([64, N], BF16, name="oh2")
        itw = big.tile([32, N], I32, name="itw")
        ith = big.tile([32, N], I32, name="ith")
        nc.gpsimd.iota(itw[:].rearrange("p (mx my) -> p mx my", mx=32),
                       pattern=[[0, 32], [1, 32]], base=0, channel_multiplier=-1)
        nc.gpsimd.iota(ith[:].rearrange("p (mx my) -> p mx my", mx=32),
                       pattern=[[1, 32], [0, 32]], base=0, channel_multiplier=-1)
        nc.vector.tensor_scalar(oh2[0:32, :], itw[:], 0, None, ALU.is_equal)
        nc.vector.tensor_scalar(oh2[32:64, :], ith[:], 0, None, ALU.is_equal)

        # per-partition bias AP for the ACT exp share
        bias_t = big.tile([128, 1], F32, name="bias_t")
        nc.vector.memset(bias_t[:, :], ACT_BIAS)

        # V natural layout + ones column -> vp [128, (mchunk, head, 33)] bf16
        vp = big.tile([128, 8 * NH * 33], BF16, name="vp")
        vp_r = vp[:].rearrange("p (t h c) -> p t h c", t=8, h=NH)
        xv_r = xv[:].rearrange("p (t h c) -> p t h c", t=8, h=NH)
        nc.vector.tensor_copy(vp_r[:, :, :, 0:32], xv_r)
        nc.vector.memset(vp_r[:, :, :, 32:33], 1.0)

        # extended operands, one 1024-col block per head (row layout chosen so
        # the 64-wide B-copies start at partition 0):
        #   ke rows 0:32 Aw  | 32:64 Ah  | 64:96 K^T_h
        #   qe rows 0:32 Bw_h | 32:64 Bh_h | 64:96 Q^T_h
        ke = big.tile([96, NH * N], BF16, name="ke")
        qe = big.tile([96, NH * N], BF16, name="qe")
        for h in range(NH):
            qt = qt0 if h < 4 else qt1
            kt = kt0 if h < 4 else kt1
            p0 = (h % 4) * 32
            cs = slice(h * N, (h + 1) * N)
            nc.sync.dma_start(out=qe[64:96, cs], in_=qt[p0:p0 + 32, :])
            nc.sync.dma_start(out=ke[64:96, cs], in_=kt[p0:p0 + 32, :])
            nc.sync.dma_start(out=ke[0:64, cs], in_=oh2[:, :])

        # ================= B rows (Bw/Bh) =================
        # Per head-pair (2j, 2j+1): b_ps [128, 1024] holds
        #   partitions  0:32  Bw(2j)   |  32:64  Bh(2j)
        #   partitions 64:96  Bw(2j+1) | 96:128  Bh(2j+1)
        # free = (y, x) GLOBAL position y*32+x: w-MMs (fixed y) write 32
        # contiguous cols; h-MMs (fixed x) write 32 cols at stride 32.  This
        # makes the PSUM->qe copy a single uniform [64, 1024] AP per head.
        # Row-group of head h is 32*(h%4) and matches where Q^T_h lives in
        # qt0/qt1; four 32x32 array tiles run concurrently.
        #   Bw[y',n]|y(n)=y = rel_w[31-y+y'] . Q[n] -> lhsT = rt128[., 31-y:63-y]
        #   Bh[x',n]|x(n)=x = rel_h[31-x+x'] . Q[n] -> lhsT = rt128[., 95-x:127-x]
        qe_v = qe[:].rearrange("p (h nx ny) -> p h nx ny", h=NH, nx=32)

        def emit_b_pair(j):
            qt = qt0 if j < 2 else qt1
            b_ps = psp.tile([128, 1024], F32, name="ps")
            for y in range(32):
                for sub in range(2):
                    h = 2 * j + sub
                    hh = h % 4
                    qv = qt[:].rearrange("p (nx ny) -> p nx ny", nx=32)
                    rhs_w = qv[32 * hh:32 * hh + 32, :, y:y + 1]
                    rhs_h = qv[32 * hh:32 * hh + 32, y:y + 1, :]
                    # w rows: free layout (y, x); h rows: free layout (x, y)=n.
                    # Both MM kinds write one contiguous 32-col block (1 bank).
                    nc.tensor.matmul(
                        b_ps[sub * 64:sub * 64 + 32, y * 32:y * 32 + 32],
                        rt128[32 * hh:32 * hh + 32, 31 - y:63 - y],
                        rhs_w, start=True, stop=True,
                        tile_position=(32 * hh, sub * 64),
                    )
                    nc.tensor.matmul(
                        b_ps[sub * 64 + 32:sub * 64 + 64, y * 32:y * 32 + 32],
                        rt128[32 * hh:32 * hh + 32, 95 - y:127 - y],
                        rhs_h, start=True, stop=True,
                        tile_position=(32 * hh, sub * 64 + 32),
                    )
            # PSUM -> qe[0:32]/[32:64]: w needs a (y,x)->(x,y) scatter --
            # that must go on DVE (1x regardless of stride; ACT is ~4.6x
            # slower on strided APs).  The contiguous h copy goes on ACT.
            for sub in range(2):
                h = 2 * j + sub
                dvw = qe_v[0:32, h, :, :].rearrange("p nx ny -> p ny nx")
                srcw = b_ps[sub * 64:sub * 64 + 32, :].rearrange(
                    "p (y x) -> p y x", y=32)
                nc.vector.tensor_copy(dvw, srcw)
                nc.scalar.copy(
                    qe_v[32:64, h, :, :],
                    b_ps[sub * 64 + 32:sub * 64 + 64, :].rearrange(
                        "p (x y) -> p x y", x=32),
                )

        # ================= main loop =================
        # Emission order keeps PE streaming: QK+exp of head h, then AV +
        # normalize of head h-1.
        out_r = out.rearrange("(j p) c -> p j c", p=128)
        pts_by_head = {}
        a_by_head = {}

        def emit_qk_exp(h):
            dve_set = DVE_EXP.get(h, DVE_EXP_DEFAULT)
            pts = []
            for i in range(8):
                l_ps = psp.tile([128, N], F32, name="ps")
                for c in range(2):
                    nc.tensor.matmul(
                        l_ps[:, c * 512:(c + 1) * 512],
                        ke[:, h * N + i * 128: h * N + i * 128 + 128],
                        qe[:, h * N + c * 512: h * N + (c + 1) * 512],
                        start=True, stop=True,
                    )
                pt = ptp.tile([128, N], BF16, name="pt")
                if i in dve_set:
                    nc.vector.tensor_scalar(
                        pt[:].bitcast(I16), l_ps[:], SCH_A, SCH_B,
                        ALU.mult, ALU.add,
                    )
                else:
                    nc.scalar.activation(pt[:], l_ps[:], AF.Exp,
                                         bias=bias_t[:, :], scale=EXP_SCALE)
                pts.append(pt)
            pts_by_head[h] = pts

        def emit_av_norm(h):
            pts = pts_by_head.pop(h)
            a_full = psp.tile([128, 1024], F32, name="ps")
            a_ps = a_full[:, 0:288]
            # one accumulation group per bank at a time, hence j outer
            for j in range(8):
                for i in range(8):
                    nc.tensor.matmul(
                        a_ps[:, j * 36: j * 36 + 33],
                        pts[i][:, j * 128:(j + 1) * 128],
                        vp[:, (i * NH + h) * 33: (i * NH + h) * 33 + 33],
                        start=(i == 0), stop=(i == 7),
                    )
            a_r = a_ps.rearrange("p (j c) -> p j c", c=36)
            r = outp.tile([128, 8], F32, name="r")
            r_r = r[:].rearrange("p (j o) -> p j o", o=1)
            nc.vector.reciprocal(r_r, a_r[:, :, 32:33])
            o_sb = outp.tile([128, 256], F32, name="o_sb")
            o_r = o_sb[:].rearrange("p (j c) -> p j c", c=32)
            nc.vector.tensor_tensor(
                out=o_r, in0=a_r[:, :, 0:32],
                in1=r_r.broadcast_to((128, 8, 32)), op=ALU.mult,
            )
            nc.sync.dma_start(out=out_r[:, :, h * 32:(h + 1) * 32], in_=o_r)

        emit_b_pair(0)
        for h in range(NH):
            if h % 2 == 1 and h < 7:
                emit_b_pair((h + 1) // 2)
            emit_qk_exp(h)
            if h > 0:
                emit_av_norm(h - 1)
        emit_av_norm(NH - 1)


def build_nc():
    if "nc" in _CACHE:
        return _CACHE["nc"]
    nc = bacc.Bacc(
        "TRN2", target_bir_lowering=False, debug=False, num_devices=N_CORES
    )
    x = nc.dram_tensor("x", [N, 768], F32, kind="ExternalInput")
    rw = nc.dram_tensor("rw", [63, 32], F32, kind="ExternalInput")
    rh = nc.dram_tensor("rh", [63, 32], F32, kind="ExternalInput")
    out = nc.dram_tensor("out", [N, 256], F32, kind="ExternalOutput")
    with TileContext(nc) as tc:
        _emit(tc, x.ap(), rw.ap(), rh.ap(), out.ap())
    nc.compile()
    _CACHE["nc"] = nc
    return nc


def kernel(inputs, key_rel_w, key_rel_h):
    B = inputs.shape[0]
    assert inputs.shape == (8, 32, 32, 768), inputs.shape
    nc = build_nc()
    x_full = np.ascontiguousarray(inputs.reshape(B, N, 768), dtype=np.float32)
    rw = np.ascontiguousarray(key_rel_w, dtype=np.float32)
    rh = np.ascontiguousarray(key_rel_h, dtype=np.float32)
    in_maps = [{"x": x_full[b], "rw": rw, "rh": rh} for b in range(N_CORES)]
    res = run_bass_kernel_spmd(nc, in_maps, list(range(N_CORES)))
    return np.stack(
        [res.results[b]["out"].reshape(32, 32, 256) for b in range(N_CORES)]
    )


if __name__ == "__main__":
    rng = np.random.default_rng(0)
    inputs = rng.standard_normal((8, 32, 32, 768), dtype=np.float32)
    rw = rng.standard_normal((63, 32), dtype=np.float32) * 32 ** -0.5
    rh = rng.standard_normal((63, 32), dtype=np.float32) * 32 ** -0.5
    o = kernel(inputs, rw, rh)
    print(o.shape, o.dtype, float(np.abs(o).max()))


# revision 15
# speedup vs baseline: 1.0924x; 1.0924x over previous
"""Trainium2 Bass kernel for 2D attention with relative-position augmentation.

Problem shapes (hardcoded): inputs [8, 32, 32, 768] fp32 (q|k|v packed on the
channel axis, 256 each), key_rel_w/key_rel_h [63, 32] fp32.
Output: [8, 32, 32, 256] fp32.

Sharding: data-parallel over batch - core b gets batch b (8 cores, no
collectives needed).

Per-core math (N = 32*32 = 1024 tokens, 8 heads, head dim 32):
  L[n, m] = Q[n].K[m] + qdw[n, y2(m)-y(n)+31] + qdh[n, x2(m)-x(n)+31]
  out[n]  = softmax_m(L[n, :] / sqrt(32)) @ V
where qdw = Q @ key_rel_w^T, qdh = Q @ key_rel_h^T and n=(x,y), m=(x2,y2).

Kernel formulation (v2):
  * L^T (m on partitions, n on free dim) so the attention matmul consumes
    P^T = exp(L^T) directly as the stationary operand.
  * Relative-logit terms folded into the SAME matmul as Q.K by extending the
    contraction dim from 32 to 96 (rows 32:64 one-hot Aw / Bw, 64:96 Ah / Bh).
    The Bw/Bh rows (partition-shifted qdw^T/qdh^T) are produced by per-y
    matmuls with shifted free-slices of the rel tables as stationary; four
    32x32 array tiles (2 row-groups x 2 col-groups) run concurrently.
  * Softmax skips max-subtraction (logits are small); 1/sqrt(32) is folded
    into the exponential.
  * exp() is split between the Scalar engine (exact table exp) and the Vector
    engine (Schraudolph bit-trick: bits_i16 = round(x*A + B) reinterpreted as
    bf16 ~= exp(x*ES) with +-3% sawtooth error). The ACT share carries a
    bias = ln(mean sawtooth ratio) so both shares have the same mean weight
    scale and the softmax ratio cancels the systematic part.
  * Row sums via a ones column appended to V inside the AV matmul;
    normalization is one reciprocal + one broadcast tensor_tensor multiply.
"""

import numpy as np

import concourse.bacc as bacc
import concourse.mybir as mybir
from concourse.tile import TileContext
from concourse.bass_utils import run_bass_kernel_spmd

F32 = mybir.dt.float32
BF16 = mybir.dt.bfloat16
I16 = mybir.dt.int16
I32 = mybir.dt.int32
AF = mybir.ActivationFunctionType
ALU = mybir.AluOpType

N_CORES = 8
N = 1024          # tokens per batch (32 x 32)
NH = 8            # heads
EXP_SCALE = float(1.0 / np.sqrt(32.0))

# Two-term Schraudolph: s1 = i16(round(x*A + B1)) viewed as bf16 is
# (1/2)*2^(t - sig/128) * g(f) with g(f) = (1+f)*2^-f (sawtooth, mean
# 1.0406844); s2 = s1 + 64 (integer add commutes with the rounding) is the
# half-period-shifted term.  pt = s2/sqrt(2) + s1 has relative error
# h(f)/mean in [-1.0%, +0.5%] instead of +-3%, and sig is chosen so the
# mean ratio is exactly 1 (matches the exact-exp tiles; softmax cancels the
# mean).
SCH_SIGMA = float(128.0 * np.log2(1.0406844))
SCH_A = float(128.0 * np.log2(np.e) * EXP_SCALE)
SCH_B1 = float(16256.0 - SCH_SIGMA - 128.0)
SCH_C = float(2.0 ** -0.5)

# which l_ps tiles (i in 0..8) go to the Vector engine's two-term exp, per
# head; the rest go to ScalarE (the overall pace-setter).
DVE_EXP = {}
DVE_EXP_DEFAULT_EVEN = (4,)
DVE_EXP_DEFAULT_ODD = (2, 6)

_CACHE = {}


def _emit(tc, x, rw, rh, out):
    nc = tc.nc

    with tc.tile_pool(name="big", bufs=1) as big, \
         tc.tile_pool(name="dram", bufs=1, space="DRAM") as dram, \
         tc.tile_pool(name="psp", bufs=4, space="PSUM") as psp, \
         tc.tile_pool(name="ptp", bufs=17) as ptp, \
         tc.tile_pool(name="schp", bufs=4) as schp, \
         tc.tile_pool(name="outp", bufs=4) as outp:

        # ================= setup =================
        # natural-layout loads: n = t*128 + p
        xn = x.rearrange("(t p) c -> p t c", p=128)
        xq = big.tile([128, 2048], F32, name="xq")
        xk = big.tile([128, 2048], F32, name="xk")
        xv = big.tile([128, 2048], F32, name="xv")
        nc.sync.dma_start(out=xq[:].rearrange("p (t c) -> p t c", c=256),
                          in_=xn[:, :, 0:256])
        nc.sync.dma_start(out=xk[:].rearrange("p (t c) -> p t c", c=256),
                          in_=xn[:, :, 256:512])
        nc.sync.dma_start(out=xv[:].rearrange("p (t c) -> p t c", c=256),
                          in_=xn[:, :, 512:768])

        # Q,K: DVE cast fp32->bf16, store to HBM, xbar transpose-load
        qbf = big.tile([128, 2048], BF16, name="qbf")
        kbf = big.tile([128, 2048], BF16, name="kbf")
        nc.vector.tensor_copy(qbf[:], xq[:])
        nc.vector.tensor_copy(kbf[:], xk[:])
        qbf_d = dram.tile([N, 256], BF16, name="qbf_d")
        kbf_d = dram.tile([N, 256], BF16, name="kbf_d")
        nc.sync.dma_start(out=qbf_d[:].rearrange("(t p) c -> p t c", p=128),
                          in_=qbf[:].rearrange("p (t c) -> p t c", c=256))
        nc.sync.dma_start(out=kbf_d[:].rearrange("(t p) c -> p t c", p=128),
                          in_=kbf[:].rearrange("p (t c) -> p t c", c=256))
        qt0 = big.tile([128, N], BF16, name="qt0")
        qt1 = big.tile([128, N], BF16, name="qt1")
        kt0 = big.tile([128, N], BF16, name="kt0")
        kt1 = big.tile([128, N], BF16, name="kt1")
        nc.sync.dma_start(out=qt0[:], in_=qbf_d[:, 0:128], transpose=True)
        nc.sync.dma_start(out=qt1[:], in_=qbf_d[:, 128:256], transpose=True)
        nc.sync.dma_start(out=kt0[:], in_=kbf_d[:, 0:128], transpose=True)
        nc.sync.dma_start(out=kt1[:], in_=kbf_d[:, 128:256], transpose=True)


        # rel tables -> rt128 [128, 128] bf16, the 32-row table replicated at
        # 4 partition offsets (row-groups).  cols 0:63 w-table^T (col r =
        # rel_w[r, :]), col 63 zero; cols 64:127 h-table^T, col 127 zero.
        rel4 = big.tile([32, 128], F32, name="rel4")
        nc.vector.memset(rel4[:, :], 0.0)
        nc.sync.dma_start(out=rel4[0:32, 0:32], in_=rw[0:32, :])
        nc.sync.dma_start(out=rel4[0:31, 32:64], in_=rw[32:63, :])
        nc.sync.dma_start(out=rel4[0:32, 64:96], in_=rh[0:32, :])
        nc.sync.dma_start(out=rel4[0:31, 96:128], in_=rh[32:63, :])
        rtf = big.tile([32, 128], F32, name="rtf")
        nc.vector.transpose(rtf[:, :], rel4[:, :])  # 4x 32x32 block transpose
        rt128 = big.tile([128, 128], BF16, name="rt128")
        nc.vector.tensor_copy(rt128[0:32, :], rtf[:])
        for k in range(1, 4):
            nc.sync.dma_start(out=rt128[32 * k:32 * k + 32, :], in_=rt128[0:32, :])

        # one-hot rows oh2 [64, 1024]: rows 0:32 Aw[y',m]=[m%32==y'], rows
        # 32:64 Ah[x',m]=[m//32==x']
        oh2 = big.tile([64, N], BF16, name="oh2")
        itw = big.tile([32, N], I32, name="itw")
        ith = big.tile([32, N], I32, name="ith")
        nc.gpsimd.iota(itw[:].rearrange("p (mx my) -> p mx my", mx=32),
                       pattern=[[0, 32], [1, 32]], base=0, channel_multiplier=-1)
        nc.gpsimd.iota(ith[:].rearrange("p (mx my) -> p mx my", mx=32),
                       pattern=[[1, 32], [0, 32]], base=0, channel_multiplier=-1)
        nc.vector.tensor_scalar(oh2[0:32, :], itw[:], 0, None, ALU.is_equal)
        nc.vector.tensor_scalar(oh2[32:64, :], ith[:], 0, None, ALU.is_equal)

        # V natural layout + ones column -> vp [128, (mchunk, head, 33)] bf16
        vp = big.tile([128, 8 * NH * 33], BF16, name="vp")
        vp_r = vp[:].rearrange("p (t h c) -> p t h c", t=8, h=NH)
        xv_r = xv[:].rearrange("p (t h c) -> p t h c", t=8, h=NH)
        nc.vector.tensor_copy(vp_r[:, :, :, 0:32], xv_r)
        nc.vector.memset(vp_r[:, :, :, 32:33], 1.0)

        # extended operands, one 1024-col block per head (row layout chosen so
        # the 64-wide B-copies start at partition 0):
        #   ke rows 0:32 Aw  | 32:64 Ah  | 64:96 K^T_h
        #   qe rows 0:32 Bw_h | 32:64 Bh_h | 64:96 Q^T_h
        ke = big.tile([96, NH * N], BF16, name="ke")
        qe = big.tile([96, NH * N], BF16, name="qe")
        for h in range(NH):
            qt = qt0 if h < 4 else qt1
            kt = kt0 if h < 4 else kt1
            p0 = (h % 4) * 32
            cs = slice(h * N, (h + 1) * N)
            nc.sync.dma_start(out=qe[64:96, cs], in_=qt[p0:p0 + 32, :])
            nc.sync.dma_start(out=ke[64:96, cs], in_=kt[p0:p0 + 32, :])
            nc.sync.dma_start(out=ke[0:64, cs], in_=oh2[:, :])

        # ================= B rows (Bw/Bh) =================
        # Per head-pair (2j, 2j+1): b_ps [128, 1024] holds
        #   partitions  0:32  Bw(2j)   |  32:64  Bh(2j)
        #   partitions 64:96  Bw(2j+1) | 96:128  Bh(2j+1)
        # free = (y, x) GLOBAL position y*32+x: w-MMs (fixed y) write 32
        # contiguous cols; h-MMs (fixed x) write 32 cols at stride 32.  This
        # makes the PSUM->qe copy a single uniform [64, 1024] AP per head.
        # Row-group of head h is 32*(h%4) and matches where Q^T_h lives in
        # qt0/qt1; four 32x32 array tiles run concurrently.
        #   Bw[y',n]|y(n)=y = rel_w[31-y+y'] . Q[n] -> lhsT = rt128[., 31-y:63-y]
        #   Bh[x',n]|x(n)=x = rel_h[31-x+x'] . Q[n] -> lhsT = rt128[., 95-x:127-x]
        qe_v = qe[:].rearrange("p (h nx ny) -> p h nx ny", h=NH, nx=32)

        def emit_b_pair(j):
            qt = qt0 if j < 2 else qt1
            b_ps = psp.tile([128, 1024], F32, name="ps")
            for y in range(32):
                for sub in range(2):
                    h = 2 * j + sub
                    hh = h % 4
                    qv = qt[:].rearrange("p (nx ny) -> p nx ny", nx=32)
                    rhs_w = qv[32 * hh:32 * hh + 32, :, y:y + 1]
                    rhs_h = qv[32 * hh:32 * hh + 32, y:y + 1, :]
                    # w rows: free layout (y, x); h rows: free layout (x, y)=n.
                    # Both MM kinds write one contiguous 32-col block (1 bank).
                    nc.tensor.matmul(
                        b_ps[sub * 64:sub * 64 + 32, y * 32:y * 32 + 32],
                        rt128[32 * hh:32 * hh + 32, 31 - y:63 - y],
                        rhs_w, start=True, stop=True,
                        tile_position=(32 * hh, sub * 64),
                    )
                    nc.tensor.matmul(
                        b_ps[sub * 64 + 32:sub * 64 + 64, y * 32:y * 32 + 32],
                        rt128[32 * hh:32 * hh + 32, 95 - y:127 - y],
                        rhs_h, start=True, stop=True,
                        tile_position=(32 * hh, sub * 64 + 32),
                    )
            # PSUM -> qe[0:32]/[32:64], both on DVE (1x regardless of
            # stride; ACT is ~4.6x slower on strided APs and is the exp
            # pace-setter anyway).
            for sub in range(2):
                h = 2 * j + sub
                dvw = qe_v[0:32, h, :, :].rearrange("p nx ny -> p ny nx")
                srcw = b_ps[sub * 64:sub * 64 + 32, :].rearrange(
                    "p (y x) -> p y x", y=32)
                nc.vector.tensor_copy(dvw, srcw)
                nc.vector.tensor_copy(
                    qe_v[32:64, h, :, :],
                    b_ps[sub * 64 + 32:sub * 64 + 64, :].rearrange(
                        "p (x y) -> p x y", x=32),
                )

        # ================= main loop =================
        # Emission order keeps PE streaming: QK+exp of head h, then AV +
        # normalize of head h-1.
        out_r = out.rearrange("(j p) c -> p j c", p=128)
        pts_by_head = {}
        a_by_head = {}

        def emit_qk_exp(h):
            dve_set = DVE_EXP.get(
                h, DVE_EXP_DEFAULT_EVEN if h % 2 == 0 else DVE_EXP_DEFAULT_ODD)
            pts = []
            for i in range(8):
                l_ps = psp.tile([128, N], F32, name="ps")
                for c in range(2):
                    nc.tensor.matmul(
                        l_ps[:, c * 512:(c + 1) * 512],
                        ke[:, h * N + i * 128: h * N + i * 128 + 128],
                        qe[:, h * N + c * 512: h * N + (c + 1) * 512],
                        start=True, stop=True,
                    )
                pt = ptp.tile([128, N], BF16, name="pt")
                if i in dve_set:
                    s1 = schp.tile([128, N], I16, name="s1")
                    s2 = schp.tile([128, N], I16, name="s2")
                    nc.vector.tensor_scalar(
                        s1[:], l_ps[:], SCH_A, SCH_B1, ALU.mult, ALU.add)
                    nc.vector.tensor_scalar(s2[:], s1[:], 64, None, ALU.add)
                    nc.vector.scalar_tensor_tensor(
                        pt[:], s2[:].bitcast(BF16), SCH_C, s1[:].bitcast(BF16),
                        ALU.mult, ALU.add)
                else:
                    nc.scalar.activation(pt[:], l_ps[:], AF.Exp,
                                         scale=EXP_SCALE)
                pts.append(pt)
            pts_by_head[h] = pts

        def emit_av_norm(h):
            pts = pts_by_head.pop(h)
            a_full = psp.tile([128, 1024], F32, name="ps")
            a_ps = a_full[:, 0:288]
            # one accumulation group per bank at a time, hence j outer
            for j in range(8):
                for i in range(8):
                    nc.tensor.matmul(
                        a_ps[:, j * 36: j * 36 + 33],
                        pts[i][:, j * 128:(j + 1) * 128],
                        vp[:, (i * NH + h) * 33: (i * NH + h) * 33 + 33],
                        start=(i == 0), stop=(i == 7),
                    )
            a_r = a_ps.rearrange("p (j c) -> p j c", c=36)
            r = outp.tile([128, 8], F32, name="r")
            r_r = r[:].rearrange("p (j o) -> p j o", o=1)
            nc.vector.reciprocal(r_r, a_r[:, :, 32:33])
            o_sb = outp.tile([128, 256], F32, name="o_sb")
            o_r = o_sb[:].rearrange("p (j c) -> p j c", c=32)
            nc.vector.tensor_tensor(
                out=o_r, in0=a_r[:, :, 0:32],
                in1=r_r.broadcast_to((128, 8, 32)), op=ALU.mult,
            )
            nc.sync.dma_start(out=out_r[:, :, h * 32:(h + 1) * 32], in_=o_r)

        emit_b_pair(0)
        for h in range(NH):
            if h % 2 == 1 and h < 7:
                emit_b_pair((h + 1) // 2)
            emit_qk_exp(h)
            if h > 0:
                emit_av_norm(h - 1)
        emit_av_norm(NH - 1)


def build_nc():
    if "nc" in _CACHE:
        return _CACHE["nc"]
    nc = bacc.Bacc(
        "TRN2", target_bir_lowering=False, debug=False, num_devices=N_CORES
    )
    x = nc.dram_tensor("x", [N, 768], F32, kind="ExternalInput")
    rw = nc.dram_tensor("rw", [63, 32], F32, kind="ExternalInput")
    rh = nc.dram_tensor("rh", [63, 32], F32, kind="ExternalInput")
    out = nc.dram_tensor("out", [N, 256], F32, kind="ExternalOutput")
    with TileContext(nc) as tc:
        _emit(tc, x.ap(), rw.ap(), rh.ap(), out.ap())
    nc.compile()
    _CACHE["nc"] = nc
    return nc


def kernel(inputs, key_rel_w, key_rel_h):
    B = inputs.shape[0]
    assert inputs.shape == (8, 32, 32, 768), inputs.shape
    nc = build_nc()
    x_full = np.ascontiguousarray(inputs.reshape(B, N, 768), dtype=np.float32)
    rw = np.ascontiguousarray(key_rel_w, dtype=np.float32)
    rh = np.ascontiguousarray(key_rel_h, dtype=np.float32)
    in_maps = [{"x": x_full[b], "rw": rw, "rh": rh} for b in range(N_CORES)]
    res = run_bass_kernel_spmd(nc, in_maps, list(range(N_CORES)))
    return np.stack(
        [res.results[b]["out"].reshape(32, 32, 256) for b in range(N_CORES)]
    )


if __name__ == "__main__":
    rng = np.random.default_rng(0)
    inputs = rng.standard_normal((8, 32, 32, 768), dtype=np.float32)
    rw = rng.standard_normal((63, 32), dtype=np.float32) * 32 ** -0.5
    rh = rng.standard_normal((63, 32), dtype=np.float32) * 32 ** -0.5
    o = kernel(inputs, rw, rh)
    print(o.shape, o.dtype, float(np.abs(o).max()))
